# revision 1
# baseline (speedup 1.0000x reference)
"""Cost-sensitive focal NLL loss on 8 Trainium2 NeuronCores.

Computes, for feature [N, C] logits and label [N] int:
    log_p = log_softmax(feature, axis=1)
    p = exp(log_p); beta = (1 - p)**2
    counts = bincount(label, C); ni = counts[label]; r = ni / N
    alpha = exp(r - 1) / r
    loss = -mean(alpha * beta[i, label[i]] * log_p[i, label[i]])

Only the label-column of log_p/beta is needed, so each core streams its
batch shard once, computes per-row sumexp on ScalarE (fused accumulate),
builds one-hot masks on VectorE for the class histogram (summed across
rows on TensorE), gathers the label logits with an indirect DMA, and
AllReduces the per-class counts across the 8 cores. Each core emits one
partial sum; the host adds the 8 partials and divides by -N.
"""

import os

import numpy as np

import concourse.bacc as bacc
import concourse.bass as bass
import concourse.mybir as mybir
import concourse.tile as tile
from concourse.bass_utils import run_bass_kernel_spmd

N_CORES = 8
N = 16384
C = 1000
P = 128
ROWS = N // N_CORES          # 2048 rows per core
T = ROWS // P                # 16 row-tiles per core
CH0 = 500                    # class chunk (PSUM matmul free dim <= 512)

FP = mybir.dt.float32
I32 = mybir.dt.int32

LAST_RESULTS = None  # BassKernelResults of the most recent run (for profiling)


def build_program(
    dump_debug: bool = False,
    xgather_multi: bool = True,
    reps: int = 1,
    single_core: bool = False,
    ablate: frozenset = frozenset(),
):
    nc = bacc.Bacc(
        "TRN2",
        target_bir_lowering=False,
        debug=False,
        enable_asserts=False,
        num_devices=1 if single_core else N_CORES,
    )
    nc._single_core_variant = single_core
    nc._ablate = ablate

    feature = nc.dram_tensor("feature", [ROWS, C], FP, kind="ExternalInput")
    label_cm = nc.dram_tensor("label_cm", [P, T], I32, kind="ExternalInput")
    out = nc.dram_tensor("out", [1, 1], FP, kind="ExternalOutput")
    dbg = {}
    if dump_debug:
        for nm, shape in [
            ("d_s", [P, T]),
            ("d_x", [P, T]),
            ("d_u", [P, T]),
            ("d_cnt_local", [1, C]),
            ("d_cnt_global", [1, C]),
            ("d_U", [1, C]),
            ("d_A", [1, C]),
        ]:
            dbg[nm] = nc.dram_tensor(nm, shape, FP, kind="ExternalOutput")

    with tile.TileContext(nc) as tc:
        with (
            tc.tile_pool(name="const", bufs=1) as const_pool,
            tc.tile_pool(name="feat", bufs=4) as feat_pool,
            tc.tile_pool(name="mask", bufs=1) as mask_pool,
            tc.tile_pool(name="escr", bufs=2) as escr_pool,
            tc.tile_pool(name="small", bufs=1) as small_pool,
            tc.tile_pool(name="psum", bufs=1, space="PSUM") as psum_pool,
            tc.tile_pool(name="dram", bufs=1, space="DRAM") as dram_pool,
        ):
            # ---- constants ----
            iota_i = const_pool.tile([P, C], I32)
            nc.gpsimd.iota(iota_i[:], pattern=[[1, C]], base=0, channel_multiplier=0)
            iota_f = const_pool.tile([P, C], FP)
            nc.vector.tensor_copy(iota_f[:], iota_i[:])

            ones_col = const_pool.tile([P, 1], FP)
            nc.vector.memset(ones_col[:], 1.0)
            neg1_col = const_pool.tile([P, 1], FP)
            nc.vector.memset(neg1_col[:], -1.0)

            # rowidx[p, t] = t*P + p: local row index (iota steps must fit i16)
            rowidx = const_pool.tile([P, T], I32)
            nc.gpsimd.iota(
                rowidx[:], pattern=[[P, T]], base=0, channel_multiplier=1
            )

            # ---- labels ----
            lab_i = small_pool.tile([P, T], I32)
            nc.sync.dma_start(lab_i[:], label_cm.ap())
            lab_f = small_pool.tile([P, T], FP)
            nc.vector.tensor_copy(lab_f[:], lab_i[:])

            offs = small_pool.tile([P, T], I32)  # rowidx*C + label
            nc.vector.scalar_tensor_tensor(
                offs[:],
                in0=rowidx[:],
                scalar=float(C),
                in1=lab_i[:],
                op0=mybir.AluOpType.mult,
                op1=mybir.AluOpType.add,
            )

            lnN1 = const_pool.tile([1, 1], FP)
            nc.vector.memset(lnN1[:], float(np.log(N) - 1.0))

            emit_rep_body(
                nc,
                feature,
                out,
                dbg,
                dump_debug,
                xgather_multi,
                reps,
                pools=(feat_pool, mask_pool, escr_pool, small_pool, psum_pool,
                       dram_pool),
                consts=(iota_f, ones_col, neg1_col, lnN1, lab_i, lab_f, offs),
            )

    nc.compile()
    return nc


def emit_rep_body(
    nc, feature, out, dbg, dump_debug, xgather_multi, reps, pools, consts
):
    (feat_pool, mask_pool, escr_pool, small_pool, psum_pool, dram_pool) = pools
    (iota_f, ones_col, neg1_col, lnN1, lab_i, lab_f, offs) = consts
    for _rep in range(reps):
        if True:
            # gather x[p, t] = feature[t*P + p, label[t*P + p]]
            # HW indirect gather uses ONE offset per dest partition row, so
            # issue one [128,1] gather per row-tile column.
            feat_flat = feature.ap().rearrange("a b -> (a b)")[:, None]
            xg = small_pool.tile([P, T], FP)
            abl = getattr(nc, "_ablate", frozenset())
            if "xgather" in abl:
                nc.vector.memset(xg[:], 1.0)
            elif xgather_multi:
                for t in range(T):
                    nc.gpsimd.indirect_dma_start(
                        out=xg[:, t : t + 1],
                        out_offset=None,
                        in_=feat_flat,
                        in_offset=bass.IndirectOffsetOnAxis(
                            ap=offs[:, t : t + 1], axis=0
                        ),
                    )
            else:
                nc.gpsimd.indirect_dma_start(
                    out=xg[:],
                    out_offset=None,
                    in_=feat_flat,
                    in_offset=bass.IndirectOffsetOnAxis(ap=offs[:], axis=0),
                )

            # ---- stream feature tiles ----
            cnt_ps = [
                psum_pool.tile([1, CH0], FP, name=f"cnt_ps{i}") for i in range(2)
            ]
            s_col = small_pool.tile([P, T], FP)  # per-row sum(exp(logits))
            masks = []

            for t in range(T):
                ft = feat_pool.tile([P, C], FP)
                nc.sync.dma_start(ft[:], feature.ap()[t * P : (t + 1) * P, :])

                if "exp" not in abl:
                    esc = escr_pool.tile([P, C], FP)
                    nc.scalar.activation(
                        esc[:],
                        ft[:],
                        mybir.ActivationFunctionType.Exp,
                        accum_out=s_col[:, t : t + 1],
                    )
                elif t == 0:
                    nc.vector.memset(s_col[:], 1000.0)

                mk = mask_pool.tile([P, C], FP, name=f"mk{t}")
                masks.append(mk)
                if "mask" not in abl:
                    nc.vector.tensor_scalar(
                        mk[:],
                        iota_f[:],
                        lab_f[:, t : t + 1],
                        None,
                        op0=mybir.AluOpType.is_equal,
                    )
                elif True:
                    nc.vector.memset(mk[:, :1], 0.0)

                if "cntmm" not in abl:
                    for ci in range(2):
                        nc.tensor.matmul(
                            cnt_ps[ci][:],
                            lhsT=ones_col[:],
                            rhs=mk[:, ci * CH0 : (ci + 1) * CH0],
                            start=(t == 0),
                            stop=(t == T - 1),
                        )
                elif t == 0:
                    for ci in range(2):
                        nc.tensor.matmul(
                            cnt_ps[ci][:],
                            lhsT=ones_col[:],
                            rhs=masks[0][:, ci * CH0 : (ci + 1) * CH0],
                            start=True,
                            stop=True,
                        )

            # ---- global class counts via AllReduce ----
            cnt_sb = small_pool.tile([1, C], FP)
            for ci in range(2):
                nc.vector.tensor_copy(
                    cnt_sb[:, ci * CH0 : (ci + 1) * CH0], cnt_ps[ci][:]
                )
            cnt_in = dram_pool.tile([C, 1], FP)
            cnt_out = dram_pool.tile([C, 1], FP)
            nc.gpsimd.dma_start(cnt_in[:].rearrange("a b -> b a"), cnt_sb[:])
            if getattr(nc, "_single_core_variant", False):
                nc.gpsimd.dma_start(cnt_out[:], cnt_in[:])
            else:
                nc.gpsimd.collective_compute(
                    "AllReduce",
                    mybir.AluOpType.add,
                    replica_groups=[list(range(N_CORES))],
                    ins=[cnt_in.opt()],
                    outs=[cnt_out.opt()],
                )

            # per-class weight A_c = exp(r - 1)/r, r = n/N, via
            # A = exp(n/N - 1 - ln(n) + ln(N))  (no reciprocal needed)
            gcnt = small_pool.tile([1, C], FP)
            nc.gpsimd.dma_start(gcnt[:], cnt_out[:].rearrange("a b -> b a"))
            # clamp to >= 0.5 so absent classes (U_c = 0) stay finite
            gcnt_c = small_pool.tile([1, C], FP)
            nc.vector.tensor_scalar_max(gcnt_c[:], gcnt[:], 0.5)
            ln_n = small_pool.tile([1, C], FP)
            nc.scalar.activation(
                ln_n[:], gcnt_c[:], mybir.ActivationFunctionType.Ln
            )
            zz = small_pool.tile([1, C], FP)  # n/N - ln(n)
            nc.vector.scalar_tensor_tensor(
                zz[:],
                in0=gcnt_c[:],
                scalar=1.0 / N,
                in1=ln_n[:],
                op0=mybir.AluOpType.mult,
                op1=mybir.AluOpType.subtract,
            )
            aa = small_pool.tile([1, C], FP)
            nc.scalar.activation(
                aa[:], zz[:], mybir.ActivationFunctionType.Exp, bias=lnN1[:]
            )

            # ---- per-row tail ([P, T] elementwise) ----
            lse = small_pool.tile([P, T], FP)
            nc.scalar.activation(lse[:], s_col[:], mybir.ActivationFunctionType.Ln)

            logp = small_pool.tile([P, T], FP)
            nc.vector.tensor_tensor(
                logp[:], xg[:], lse[:], op=mybir.AluOpType.subtract
            )

            pp = small_pool.tile([P, T], FP)
            nc.scalar.activation(pp[:], logp[:], mybir.ActivationFunctionType.Exp)

            beta = small_pool.tile([P, T], FP)  # (p-1)^2 == (1-p)^2
            nc.scalar.activation(
                beta[:], pp[:], mybir.ActivationFunctionType.Square, bias=neg1_col[:]
            )

            u = small_pool.tile([P, T], FP)
            nc.vector.tensor_tensor(u[:], beta[:], logp[:], op=mybir.AluOpType.mult)

            # U_c = sum_{i: label_i = c} u_i  via per-tile matmuls on the masks
            u_ps = [
                psum_pool.tile([1, CH0], FP, name=f"u_ps{i}") for i in range(2)
            ]
            for t in range(T):
                # matmul lhsT must be an unsliced tile (sliced weights AP
                # crashes the exec unit) -> copy the column out first
                ucol = small_pool.tile([P, 1], FP, name=f"ucol{t}")
                nc.vector.tensor_copy(ucol[:], u[:, t : t + 1])
                for ci in range(2):
                    nc.tensor.matmul(
                        u_ps[ci][:],
                        lhsT=ucol[:],
                        rhs=masks[t][:, ci * CH0 : (ci + 1) * CH0],
                        start=(t == 0),
                        stop=(t == T - 1),
                    )
            uu = small_pool.tile([1, C], FP)
            for ci in range(2):
                nc.vector.tensor_copy(uu[:, ci * CH0 : (ci + 1) * CH0], u_ps[ci][:])

            # partial = sum_c A_c * U_c  (tensor_tensor_reduce errors on HW)
            au = small_pool.tile([1, C], FP)
            nc.vector.tensor_tensor(au[:], aa[:], uu[:], op=mybir.AluOpType.mult)
            fin_sb = small_pool.tile([1, 1], FP)
            nc.vector.tensor_reduce(
                fin_sb[:], au[:], axis=mybir.AxisListType.X, op=mybir.AluOpType.add
            )
            nc.sync.dma_start(out.ap(), fin_sb[:])

            if dump_debug:
                nc.sync.dma_start(dbg["d_s"].ap(), s_col[:])
                nc.sync.dma_start(dbg["d_x"].ap(), xg[:])
                nc.sync.dma_start(dbg["d_u"].ap(), u[:])
                nc.sync.dma_start(dbg["d_cnt_local"].ap(), cnt_sb[:])
                nc.sync.dma_start(dbg["d_cnt_global"].ap(), gcnt[:])
                nc.sync.dma_start(dbg["d_U"].ap(), uu[:])
                nc.sync.dma_start(dbg["d_A"].ap(), aa[:])


_NC_CACHE = None


def _get_nc():
    global _NC_CACHE
    if _NC_CACHE is None:
        _NC_CACHE = build_program()
    return _NC_CACHE


def kernel(feature: np.ndarray, label: np.ndarray) -> np.ndarray:
    global LAST_RESULTS
    feature = np.ascontiguousarray(np.asarray(feature, dtype=np.float32))
    label = np.asarray(label)
    assert feature.shape == (N, C), feature.shape
    assert label.shape == (N,), label.shape

    lab32 = label.astype(np.int32)

    in_maps = []
    for k in range(N_CORES):
        fshard = feature[k * ROWS : (k + 1) * ROWS]
        lshard = lab32[k * ROWS : (k + 1) * ROWS]
        # column-major: [p, t] = label[t*P + p], matching row-tile partitions
        lab_cm = np.ascontiguousarray(lshard.reshape(T, P).T)
        in_maps.append(
            {"feature": np.ascontiguousarray(fshard), "label_cm": lab_cm}
        )

    nc = _get_nc()
    trace = bool(int(os.environ.get("KERNEL_TRACE", "0")))
    res = run_bass_kernel_spmd(
        nc,
        in_maps,
        core_ids=list(range(N_CORES)),
        trace=trace,
    )
    LAST_RESULTS = res

    total = 0.0
    for k in range(N_CORES):
        total += float(res.results[k]["out"][0, 0])
    return np.float32(-total / N)



# revision 9
# speedup vs baseline: 1.4533x; 1.4533x over previous
"""Cost-sensitive focal NLL loss on 8 Trainium2 NeuronCores.

For feature [N, C] logits and label [N] int:
    log_p = log_softmax(feature, axis=1)
    p = exp(log_p); beta = (1 - p)**2
    counts = bincount(label, C); ni = counts[label]; r = ni / N
    alpha = exp(r - 1) / r
    loss = -mean(alpha * beta[i, label[i]] * log_p[i, label[i]])

Only the label-column of log_p/beta is needed, so each core streams its
2048-row feature shard once (exp with fused row-sum on ScalarE) and
gathers the label logits with indirect DMAs.

The global class histogram couples all rows, but instead of an
AllReduce (which costs a cross-core barrier ~40us of launch skew plus
~20us collective latency), every core receives ALL 16384 labels (64KB,
rotated so its own shard's labels are always columns 0..15) and
computes the global histogram redundantly: decompose c = 32*hi + lo,
build one-hot masks of hi and lo per 128-label chunk, and accumulate
maskA_g^T @ maskB_g on the PE into a [128,128] PSUM tile whose four
diagonal 32x32 blocks sum to counts[hi, lo]. Zero collectives; each
core's NEFF is fully independent.

Per-class weight A = exp(r-1)/r is computed without Ln (reciprocal on
VectorE), keeping the ScalarE exp table loaded until the single ln()
needed for log-sum-exp at the tail. The per-row u = beta*logp values
are folded per-class with the same mask matmuls, and sum_c A_c * U_c
reduces to a [32,1] partial per core; the host sums 8x32 partials.
"""

import os

import numpy as np

import concourse.bacc as bacc
import concourse.bass as bass
import concourse.mybir as mybir
import concourse.tile as tile
from concourse.bass_utils import run_bass_kernel_spmd

N_CORES = 8
N = 16384
C = 1000
P = 128
ROWS = N // N_CORES          # 2048 rows per core
T = ROWS // P                # 16 row-tiles per core
Q = N // P                   # 128 label chunks of 128 (all cores' labels)
G = Q // 4                   # 32 mask groups of 4 chunks
HL = 32                      # c = 32*hi + lo, hi,lo in [0,32)

FP = mybir.dt.float32
I32 = mybir.dt.int32

LAST_RESULTS = None  # BassKernelResults of the most recent run (for profiling)


def build_program(dump_debug: bool = False):
    nc = bacc.Bacc(
        "TRN2",
        target_bir_lowering=False,
        debug=False,
        enable_asserts=False,
        num_devices=N_CORES,
    )

    feature = nc.dram_tensor("feature", [ROWS, C], FP, kind="ExternalInput")
    # all 16384 labels, column-major [p, q] = L[128q + p]; L is rotated so
    # this core's own shard labels are columns 0..T-1
    label_cm = nc.dram_tensor("label_cm", [P, Q], I32, kind="ExternalInput")
    out = nc.dram_tensor("out", [HL, 1], FP, kind="ExternalOutput")
    dbg = {}
    if dump_debug:
        for nm, shape in [
            ("d_cnt", [HL, HL]),
            ("d_A", [HL, HL]),
            ("d_U", [HL, HL]),
            ("d_s", [P, T]),
            ("d_x", [P, T]),
            ("d_u", [P, T]),
        ]:
            dbg[nm] = nc.dram_tensor(nm, shape, FP, kind="ExternalOutput")

    with tile.TileContext(nc) as tc:
        with (
            tc.tile_pool(name="const", bufs=1) as const_pool,
            tc.tile_pool(name="feat", bufs=4) as feat_pool,
            tc.tile_pool(name="mask", bufs=1) as mask_pool,
            tc.tile_pool(name="escr", bufs=2) as escr_pool,
            tc.tile_pool(name="small", bufs=1) as small_pool,
            tc.tile_pool(name="psum", bufs=1, space="PSUM") as psum_pool,
        ):
            # ---- labels + constants ----
            lab_i = small_pool.tile([P, Q], I32)
            nc.sync.dma_start(lab_i[:], label_cm.ap())

            iota32_i = const_pool.tile([P, HL], I32)
            nc.gpsimd.iota(iota32_i[:], pattern=[[1, HL]], base=0,
                           channel_multiplier=0)
            iota32_f = const_pool.tile([P, HL], FP)
            nc.vector.tensor_copy(iota32_f[:], iota32_i[:])

            neg1_col = const_pool.tile([P, 1], FP)
            nc.vector.memset(neg1_col[:], -1.0)

            # rowidx[p, t] = t*P + p: local row index of own shard
            rowidx = const_pool.tile([P, T], I32)
            nc.gpsimd.iota(rowidx[:], pattern=[[P, T]], base=0,
                           channel_multiplier=1)

            # offs[p, t] = rowidx*C + own_label (own labels = cols 0..T-1)
            offs = small_pool.tile([P, T], I32)
            nc.vector.scalar_tensor_tensor(
                offs[:],
                in0=rowidx[:],
                scalar=float(C),
                in1=lab_i[:, 0:T],
                op0=mybir.AluOpType.mult,
                op1=mybir.AluOpType.add,
            )

            # hi = label >> 5, lo = label & 31, as fp32 for is_equal masks
            hi_i = small_pool.tile([P, Q], I32)
            nc.vector.tensor_scalar(
                hi_i[:], lab_i[:], 5, None,
                op0=mybir.AluOpType.logical_shift_right,
            )
            lo_i = small_pool.tile([P, Q], I32)
            nc.vector.tensor_scalar(
                lo_i[:], lab_i[:], 31, None,
                op0=mybir.AluOpType.bitwise_and,
            )
            hi_f = small_pool.tile([P, Q], FP)
            nc.vector.tensor_copy(hi_f[:], hi_i[:])
            lo_f = small_pool.tile([P, Q], FP)
            nc.vector.tensor_copy(lo_f[:], lo_i[:])

            # ---- x-gathers: x[p, t] = feature[t*P + p, label[t*P + p]] ----
            feat_flat = feature.ap().rearrange("a b -> (a b)")[:, None]
            xg = small_pool.tile([P, T], FP)
            for t in range(T):
                nc.gpsimd.indirect_dma_start(
                    out=xg[:, t : t + 1],
                    out_offset=None,
                    in_=feat_flat,
                    in_offset=bass.IndirectOffsetOnAxis(
                        ap=offs[:, t : t + 1], axis=0
                    ),
                )

            # ---- feature stream: exp + fused row-sum on ScalarE ----
            # (emitted before masks so the first DMAs/exps start immediately;
            # mask building below only occupies VectorE/TensorE)
            s_col = small_pool.tile([P, T], FP)  # per-row sum(exp(logits))
            fts = []
            for t in range(T):
                ft = feat_pool.tile([P, C], FP, name="ft")
                nc.sync.dma_start(ft[:], feature.ap()[t * P : (t + 1) * P, :])
                fts.append(ft)
            for t in range(T):
                esc = escr_pool.tile([P, C], FP, name="esc")
                nc.scalar.activation(
                    esc[:],
                    fts[t][:],
                    mybir.ActivationFunctionType.Exp,
                    accum_out=s_col[:, t : t + 1],
                )

            # ---- one-hot masks for the global histogram ----
            # maskB_all[p, q, j] = (lo(label[p, q]) == j)
            maskB = mask_pool.tile([P, Q, HL], FP)
            nc.vector.tensor_tensor(
                maskB[:],
                lo_f[:].unsqueeze(2).broadcast_to([P, Q, HL]),
                iota32_f[:].unsqueeze(1).broadcast_to([P, Q, HL]),
                op=mybir.AluOpType.is_equal,
            )
            # maskA_g[p, k, h] = (hi(label[p, 4g+k]) == h), one tile per
            # group of 4 chunks (matmul lhsT must be an unsliced tile)
            maskAs = []
            for g in range(G):
                mA = mask_pool.tile([P, 4, HL], FP, name=f"mA{g}")
                maskAs.append(mA)
                nc.vector.tensor_tensor(
                    mA[:],
                    hi_f[:, 4 * g : 4 * g + 4].unsqueeze(2).broadcast_to(
                        [P, 4, HL]
                    ),
                    iota32_f[:].unsqueeze(1).broadcast_to([P, 4, HL]),
                    op=mybir.AluOpType.is_equal,
                )

            # hist_ps[32k+h, 32k'+j] += sum_p maskA_g[p,k,h]*maskB_g[p,k',j]
            # diagonal blocks k==k' hold per-chunk histograms
            hist_ps = psum_pool.tile([P, P], FP)
            for g in range(G):
                nc.tensor.matmul(
                    hist_ps[:],
                    lhsT=maskAs[g][:],
                    rhs=maskB[:, 4 * g : 4 * g + 4, :],
                    start=(g == 0),
                    stop=(g == G - 1),
                )

            # counts[h, j] = sum_k hist_ps[32k+h, 32k+j]. Vector engines have
            # no cross-lane path, so shift the diagonal blocks onto partitions
            # 0..31 with SBUF->SBUF DMAs (PSUM is not DMA-able: copy out first)
            hist_sb = small_pool.tile([P, P], FP)
            nc.vector.tensor_copy(hist_sb[:], hist_ps[:])
            cdiag = small_pool.tile([HL, 4, HL], FP)
            for k in range(4):
                nc.sync.dma_start(
                    cdiag[:, k, :],
                    hist_sb[k * HL : (k + 1) * HL, k * HL : (k + 1) * HL],
                )
            cab = small_pool.tile([HL, HL], FP)
            nc.vector.tensor_tensor(cab[:], cdiag[:, 0, :], cdiag[:, 1, :],
                                    op=mybir.AluOpType.add)
            ccd = small_pool.tile([HL, HL], FP)
            nc.vector.tensor_tensor(ccd[:], cdiag[:, 2, :], cdiag[:, 3, :],
                                    op=mybir.AluOpType.add)
            cnt = small_pool.tile([HL, HL], FP)
            nc.vector.tensor_tensor(cnt[:], cab[:], ccd[:],
                                    op=mybir.AluOpType.add)

            # A = exp(n/N - 1) * N/n, clamped so absent classes stay finite
            cntc = small_pool.tile([HL, HL], FP)
            nc.vector.tensor_scalar_max(cntc[:], cnt[:], 0.5)
            rn = small_pool.tile([HL, HL], FP)
            nc.vector.reciprocal(rn[:], cntc[:])

            # ---- ScalarE tail (ordered to keep the Exp table loaded) ----
            expx = small_pool.tile([P, T], FP)
            nc.scalar.activation(expx[:], xg[:],
                                 mybir.ActivationFunctionType.Exp)
            e1 = small_pool.tile([HL, HL], FP)
            nc.scalar.activation(
                e1[:], cnt[:], mybir.ActivationFunctionType.Exp,
                bias=neg1_col[0:HL, :], scale=1.0 / N,
            )
            aw = small_pool.tile([HL, HL], FP)  # A = e1 * N * (1/n)
            nc.vector.scalar_tensor_tensor(
                aw[:],
                in0=e1[:],
                scalar=float(N),
                in1=rn[:],
                op0=mybir.AluOpType.mult,
                op1=mybir.AluOpType.mult,
            )
            lse = small_pool.tile([P, T], FP)  # the only Ln: one table switch
            nc.scalar.activation(lse[:], s_col[:],
                                 mybir.ActivationFunctionType.Ln)

            # ---- per-row tail on VectorE ----
            sinv = small_pool.tile([P, T], FP)
            nc.vector.reciprocal(sinv[:], s_col[:])
            pp = small_pool.tile([P, T], FP)  # p = exp(x)/sumexp
            nc.vector.tensor_tensor(pp[:], expx[:], sinv[:],
                                    op=mybir.AluOpType.mult)
            pm1 = small_pool.tile([P, T], FP)
            nc.vector.tensor_scalar(pm1[:], pp[:], 1.0, None,
                                    op0=mybir.AluOpType.subtract)
            beta = small_pool.tile([P, T], FP)  # (p-1)^2 == (1-p)^2
            nc.vector.tensor_tensor(beta[:], pm1[:], pm1[:],
                                    op=mybir.AluOpType.mult)
            logp = small_pool.tile([P, T], FP)
            nc.vector.tensor_tensor(logp[:], xg[:], lse[:],
                                    op=mybir.AluOpType.subtract)
            u = small_pool.tile([P, T], FP)
            nc.vector.tensor_tensor(u[:], beta[:], logp[:],
                                    op=mybir.AluOpType.mult)

            # ---- U[h, j] via the same masks (own labels = chunks 0..15) ----
            u_ps = psum_pool.tile([P, P], FP)
            for g in range(4):
                uA = small_pool.tile([P, 4, HL], FP, name=f"uA{g}")
                nc.vector.tensor_tensor(
                    uA[:],
                    maskAs[g][:],
                    u[:, 4 * g : 4 * g + 4].unsqueeze(2).broadcast_to(
                        [P, 4, HL]
                    ),
                    op=mybir.AluOpType.mult,
                )
                nc.tensor.matmul(
                    u_ps[:],
                    lhsT=uA[:],
                    rhs=maskB[:, 4 * g : 4 * g + 4, :],
                    start=(g == 0),
                    stop=(g == 3),
                )
            u_sb = small_pool.tile([P, P], FP)
            nc.vector.tensor_copy(u_sb[:], u_ps[:])
            udiag = small_pool.tile([HL, 4, HL], FP)
            for k in range(4):
                nc.sync.dma_start(
                    udiag[:, k, :],
                    u_sb[k * HL : (k + 1) * HL, k * HL : (k + 1) * HL],
                )
            uab = small_pool.tile([HL, HL], FP)
            nc.vector.tensor_tensor(uab[:], udiag[:, 0, :], udiag[:, 1, :],
                                    op=mybir.AluOpType.add)
            ucd = small_pool.tile([HL, HL], FP)
            nc.vector.tensor_tensor(ucd[:], udiag[:, 2, :], udiag[:, 3, :],
                                    op=mybir.AluOpType.add)
            ublk = small_pool.tile([HL, HL], FP)
            nc.vector.tensor_tensor(ublk[:], uab[:], ucd[:],
                                    op=mybir.AluOpType.add)

            # partial[h] = sum_j A[h,j] * U[h,j]; host sums 8 x 32 partials
            au = small_pool.tile([HL, HL], FP)
            nc.vector.tensor_tensor(au[:], aw[:], ublk[:],
                                    op=mybir.AluOpType.mult)
            fin = small_pool.tile([HL, 1], FP)
            nc.vector.tensor_reduce(
                fin[:], au[:], axis=mybir.AxisListType.X,
                op=mybir.AluOpType.add,
            )
            nc.sync.dma_start(out.ap(), fin[:])

            if dump_debug:
                nc.sync.dma_start(dbg["d_cnt"].ap(), cnt[:])
                nc.sync.dma_start(dbg["d_A"].ap(), aw[:])
                nc.sync.dma_start(dbg["d_U"].ap(), ublk[:])
                nc.sync.dma_start(dbg["d_s"].ap(), s_col[:])
                nc.sync.dma_start(dbg["d_x"].ap(), xg[:])
                nc.sync.dma_start(dbg["d_u"].ap(), u[:])

    nc.compile()
    return nc


_NC_CACHE = None


def _get_nc():
    global _NC_CACHE
    if _NC_CACHE is None:
        _NC_CACHE = build_program(
            dump_debug=bool(int(os.environ.get("KERNEL_DEBUG", "0")))
        )
    return _NC_CACHE


def kernel(feature: np.ndarray, label: np.ndarray) -> np.ndarray:
    global LAST_RESULTS
    feature = np.ascontiguousarray(np.asarray(feature, dtype=np.float32))
    label = np.asarray(label)
    assert feature.shape == (N, C), feature.shape
    assert label.shape == (N,), label.shape

    lab32 = label.astype(np.int32)

    in_maps = []
    for k in range(N_CORES):
        fshard = feature[k * ROWS : (k + 1) * ROWS]
        # all labels, rotated so this core's shard occupies positions 0..2047,
        # then column-major: [p, q] = L[q*P + p]
        rot = np.concatenate([lab32[k * ROWS :], lab32[: k * ROWS]])
        lab_cm = np.ascontiguousarray(rot.reshape(Q, P).T)
        in_maps.append(
            {"feature": np.ascontiguousarray(fshard), "label_cm": lab_cm}
        )

    nc = _get_nc()
    trace = bool(int(os.environ.get("KERNEL_TRACE", "0")))
    res = run_bass_kernel_spmd(
        nc,
        in_maps,
        core_ids=list(range(N_CORES)),
        trace=trace,
    )
    LAST_RESULTS = res

    total = 0.0
    for k in range(N_CORES):
        total += float(res.results[k]["out"].sum())
    return np.float32(-total / N)


# revision 10
# speedup vs baseline: 1.8990x; 1.3067x over previous
"""Cost-sensitive focal NLL loss on 8 Trainium2 NeuronCores.

For feature [N, C] logits and label [N] int:
    log_p = log_softmax(feature, axis=1)
    p = exp(log_p); beta = (1 - p)**2
    counts = bincount(label, C); ni = counts[label]; r = ni / N
    alpha = exp(r - 1) / r
    loss = -mean(alpha * beta[i, label[i]] * log_p[i, label[i]])

Each core streams its 2048-row feature shard once through ScalarE exp
with the fused row-sum accumulator; the label-column values exp(x_i)
are then picked out of the exp outputs with a single GPSIMD
indirect_copy (an SBUF-side per-partition gather - no DMA traffic, so
the feature stream never stalls behind scattered 4-byte reads).

The global class histogram couples all rows, but instead of an
AllReduce (a cross-core barrier costs ~40us of launch skew plus ~20us
collective latency), every core receives ALL 16384 labels (128KB of
uint16, rotated so its own shard's labels are always columns 0..15)
and computes the global histogram redundantly: decompose
c = 32*hi + lo, build bf16 one-hot masks of hi and lo, and accumulate
maskA_g^T @ maskB_g on the PE into a [128,128] PSUM tile whose four
diagonal 32x32 blocks sum to counts[hi, lo]. Zero collectives; each
core's NEFF is fully independent.

A = exp(r-1)/r is computed reciprocal-style (no Ln) so ScalarE keeps
the Exp table until the single ln() at the tail (logp = ln(exp(x)/s)).
Per-row u = beta*logp folds per-class through the same masks; the
final sum_c A_c*U_c uses A pre-replicated onto the diagonal blocks
(A_tiled, zeros elsewhere) so one PSUM*SBUF multiply plus a row
reduction yields a [128,1] partial per core; the host sums 8x128.
"""

import os

import numpy as np

import concourse.bacc as bacc
import concourse.bass as bass
import concourse.mybir as mybir
import concourse.tile as tile
from concourse.bass_utils import run_bass_kernel_spmd

N_CORES = 8
N = 16384
C = 1000
P = 128
ROWS = N // N_CORES          # 2048 rows per core
T = ROWS // P                # 16 row-tiles per core
Q = N // P                   # 128 label chunks of 128 (all cores' labels)
G = Q // 4                   # 32 mask groups of 4 chunks
HL = 32                      # c = 32*hi + lo, hi,lo in [0,32)

FP = mybir.dt.float32
BF = mybir.dt.bfloat16
U16 = mybir.dt.uint16

LAST_RESULTS = None  # BassKernelResults of the most recent run (for profiling)


def build_program(dump_debug: bool = False):
    nc = bacc.Bacc(
        "TRN2",
        target_bir_lowering=False,
        debug=False,
        enable_asserts=False,
        num_devices=N_CORES,
    )

    feature = nc.dram_tensor("feature", [ROWS, C], FP, kind="ExternalInput")
    # all 16384 labels as uint16, column-major [p, q] = L[128q + p]; L is
    # rotated so this core's own shard labels are columns 0..T-1
    label_cm = nc.dram_tensor("label_cm", [P, Q], U16, kind="ExternalInput")
    out = nc.dram_tensor("out", [P, 1], FP, kind="ExternalOutput")
    dbg = {}
    if dump_debug:
        for nm, shape in [
            ("d_cnt", [HL, HL]),
            ("d_xe", [P, T]),
            ("d_s", [P, T]),
            ("d_u", [P, T]),
        ]:
            dbg[nm] = nc.dram_tensor(nm, shape, FP, kind="ExternalOutput")

    with tile.TileContext(nc) as tc:
        with (
            tc.tile_pool(name="const", bufs=1) as const_pool,
            tc.tile_pool(name="feat", bufs=4) as feat_pool,
            tc.tile_pool(name="mask", bufs=1) as mask_pool,
            tc.tile_pool(name="small", bufs=1) as small_pool,
            tc.tile_pool(name="psum", bufs=1, space="PSUM") as psum_pool,
        ):
            # ---- feature stream: issue DMAs first so HBM starts instantly
            fts = []
            for t in range(T):
                ft = feat_pool.tile([P, C], FP, name="ft")
                nc.sync.dma_start(ft[:], feature.ap()[t * P : (t + 1) * P, :])
                fts.append(ft)

            # labels on the gpsimd queue (keeps sync free for the stream)
            lab = small_pool.tile([P, Q], U16)
            nc.gpsimd.dma_start(lab[:], label_cm.ap())

            # ---- constants ----
            iota32_i = const_pool.tile([P, HL], U16)
            nc.gpsimd.iota(iota32_i[:], pattern=[[1, HL]], base=0,
                           channel_multiplier=0)
            iota32_f = const_pool.tile([P, HL], FP)
            nc.vector.tensor_copy(iota32_f[:], iota32_i[:])

            # tbase[p, t] = 1000*t (for gather indices into escall)
            tbase = const_pool.tile([P, T], U16)
            nc.gpsimd.iota(tbase[:], pattern=[[C, T]], base=0,
                           channel_multiplier=0)

            neg1_col = const_pool.tile([P, 1], FP)
            nc.vector.memset(neg1_col[:], -1.0)

            # ---- exp + fused row-sum; escall holds all 16 exp tiles ----
            s_col = small_pool.tile([P, T], FP)  # per-row sum(exp(logits))
            escall = small_pool.tile([P, T, C], FP)
            for t in range(T):
                nc.scalar.activation(
                    escall[:, t, :],
                    fts[t][:],
                    mybir.ActivationFunctionType.Exp,
                    accum_out=s_col[:, t : t + 1],
                )

            # gather exp(x_i) = escall[p, t, lab[p,t]] in one SBUF-side op
            gidx = small_pool.tile([P, T], U16)
            nc.vector.tensor_tensor(gidx[:], tbase[:], lab[:, 0:T],
                                    op=mybir.AluOpType.add)
            xe = small_pool.tile([P, T], FP)
            nc.gpsimd.indirect_copy(
                xe[:], escall[:].rearrange("p t c -> p (t c)"), gidx[:],
                i_know_ap_gather_is_preferred=True,
            )

            # ---- one-hot masks (bf16) for the global histogram ----
            hi_f = small_pool.tile([P, Q], FP)
            hi_u = small_pool.tile([P, Q], U16)
            nc.vector.tensor_scalar(
                hi_u[:], lab[:], 5, None,
                op0=mybir.AluOpType.logical_shift_right,
            )
            nc.vector.tensor_copy(hi_f[:], hi_u[:])
            lo_f = small_pool.tile([P, Q], FP)
            lo_u = small_pool.tile([P, Q], U16)
            nc.vector.tensor_scalar(
                lo_u[:], lab[:], 31, None,
                op0=mybir.AluOpType.bitwise_and,
            )
            nc.vector.tensor_copy(lo_f[:], lo_u[:])

            # maskB[p, q, j] = (lo(label[p, q]) == j)
            maskB = mask_pool.tile([P, Q, HL], BF)
            nc.vector.tensor_tensor(
                maskB[:],
                lo_f[:].unsqueeze(2).broadcast_to([P, Q, HL]),
                iota32_f[:].unsqueeze(1).broadcast_to([P, Q, HL]),
                op=mybir.AluOpType.is_equal,
            )
            # maskA_g[p, k, h] = (hi(label[p, 4g+k]) == h), one tile per
            # group of 4 chunks (matmul lhsT must be an unsliced tile)
            maskAs = []
            for g in range(G):
                mA = mask_pool.tile([P, 4, HL], BF, name=f"mA{g}")
                maskAs.append(mA)
                nc.vector.tensor_tensor(
                    mA[:],
                    hi_f[:, 4 * g : 4 * g + 4].unsqueeze(2).broadcast_to(
                        [P, 4, HL]
                    ),
                    iota32_f[:].unsqueeze(1).broadcast_to([P, 4, HL]),
                    op=mybir.AluOpType.is_equal,
                )

            # hist_ps[32k+h, 32k'+j] += sum_p maskA_g[p,k,h]*maskB_g[p,k',j]
            # diagonal blocks k==k' hold per-chunk histograms
            hist_ps = psum_pool.tile([P, P], FP)
            for g in range(G):
                nc.tensor.matmul(
                    hist_ps[:],
                    lhsT=maskAs[g][:],
                    rhs=maskB[:, 4 * g : 4 * g + 4, :],
                    start=(g == 0),
                    stop=(g == G - 1),
                )

            # counts[h, j] = sum_k hist_ps[32k+h, 32k+j]. Vector engines have
            # no cross-lane path: shift blocks onto partitions 0..31 with
            # SBUF->SBUF DMAs on the (idle) gpsimd queue.
            hist_sb = small_pool.tile([P, P], FP)
            nc.vector.tensor_copy(hist_sb[:], hist_ps[:])
            cdiag = small_pool.tile([HL, 4, HL], FP)
            for k in range(4):
                nc.gpsimd.dma_start(
                    cdiag[:, k, :],
                    hist_sb[k * HL : (k + 1) * HL, k * HL : (k + 1) * HL],
                )
            cab = small_pool.tile([HL, HL], FP)
            nc.vector.tensor_tensor(cab[:], cdiag[:, 0, :], cdiag[:, 1, :],
                                    op=mybir.AluOpType.add)
            ccd = small_pool.tile([HL, HL], FP)
            nc.vector.tensor_tensor(ccd[:], cdiag[:, 2, :], cdiag[:, 3, :],
                                    op=mybir.AluOpType.add)
            cnt = small_pool.tile([HL, HL], FP)
            nc.vector.tensor_tensor(cnt[:], cab[:], ccd[:],
                                    op=mybir.AluOpType.add)

            # A = exp(n/N - 1) * N * (1/n), n clamped at 0.5 so absent
            # classes stay finite (their U is 0)
            cntc = small_pool.tile([HL, HL], FP)
            nc.vector.tensor_scalar_max(cntc[:], cnt[:], 0.5)
            rn = small_pool.tile([HL, HL], FP)
            nc.vector.reciprocal(rn[:], cntc[:])
            e1 = small_pool.tile([HL, HL], FP)
            nc.scalar.activation(
                e1[:], cnt[:], mybir.ActivationFunctionType.Exp,
                bias=neg1_col[0:HL, :], scale=1.0 / N,
            )
            aw = small_pool.tile([HL, HL], FP)
            nc.vector.scalar_tensor_tensor(
                aw[:],
                in0=e1[:],
                scalar=float(N),
                in1=rn[:],
                op0=mybir.AluOpType.mult,
                op1=mybir.AluOpType.mult,
            )
            # A_tiled: A on the four diagonal 32x32 blocks, zero elsewhere
            # (built early, off the critical path, via gpsimd-queue DMAs)
            a_tiled = small_pool.tile([P, P], FP)
            nc.vector.memset(a_tiled[:], 0.0)
            for k in range(4):
                nc.gpsimd.dma_start(
                    a_tiled[k * HL : (k + 1) * HL, k * HL : (k + 1) * HL],
                    aw[:],
                )

            # ---- per-row tail ----
            sinv = small_pool.tile([P, T], FP)
            nc.vector.reciprocal(sinv[:], s_col[:])
            pp = small_pool.tile([P, T], FP)  # p = exp(x)/sumexp
            nc.vector.tensor_tensor(pp[:], xe[:], sinv[:],
                                    op=mybir.AluOpType.mult)
            logp = small_pool.tile([P, T], FP)  # ln(p): the only table switch
            nc.scalar.activation(logp[:], pp[:],
                                 mybir.ActivationFunctionType.Ln)
            pm1 = small_pool.tile([P, T], FP)
            nc.vector.tensor_scalar(pm1[:], pp[:], 1.0, None,
                                    op0=mybir.AluOpType.subtract)
            beta = small_pool.tile([P, T], FP)  # (p-1)^2 == (1-p)^2
            nc.vector.tensor_tensor(beta[:], pm1[:], pm1[:],
                                    op=mybir.AluOpType.mult)
            u = small_pool.tile([P, T], FP)
            nc.vector.tensor_tensor(u[:], beta[:], logp[:],
                                    op=mybir.AluOpType.mult)

            # ---- U via the same masks (own labels = chunks 0..15) ----
            u_ps = psum_pool.tile([P, P], FP)
            for g in range(4):
                uA = small_pool.tile([P, 4, HL], BF, name=f"uA{g}")
                nc.vector.tensor_tensor(
                    uA[:],
                    maskAs[g][:],
                    u[:, 4 * g : 4 * g + 4].unsqueeze(2).broadcast_to(
                        [P, 4, HL]
                    ),
                    op=mybir.AluOpType.mult,
                )
                nc.tensor.matmul(
                    u_ps[:],
                    lhsT=uA[:],
                    rhs=maskB[:, 4 * g : 4 * g + 4, :],
                    start=(g == 0),
                    stop=(g == 3),
                )

            # partial[p] = sum_j A_tiled[p,j] * u_ps[p,j]; off-diagonal
            # garbage in u_ps is zeroed by A_tiled. Host sums 8 x 128.
            au = small_pool.tile([P, P], FP)
            nc.vector.tensor_tensor(au[:], u_ps[:], a_tiled[:],
                                    op=mybir.AluOpType.mult)
            fin = small_pool.tile([P, 1], FP)
            nc.vector.tensor_reduce(
                fin[:], au[:], axis=mybir.AxisListType.X,
                op=mybir.AluOpType.add,
            )
            nc.sync.dma_start(out.ap(), fin[:])

            if dump_debug:
                nc.sync.dma_start(dbg["d_cnt"].ap(), cnt[:])
                nc.sync.dma_start(dbg["d_xe"].ap(), xe[:])
                nc.sync.dma_start(dbg["d_s"].ap(), s_col[:])
                nc.sync.dma_start(dbg["d_u"].ap(), u[:])

    nc.compile()
    return nc


_NC_CACHE = None


def _get_nc():
    global _NC_CACHE
    if _NC_CACHE is None:
        _NC_CACHE = build_program(
            dump_debug=bool(int(os.environ.get("KERNEL_DEBUG", "0")))
        )
    return _NC_CACHE


def kernel(feature: np.ndarray, label: np.ndarray) -> np.ndarray:
    global LAST_RESULTS
    feature = np.ascontiguousarray(np.asarray(feature, dtype=np.float32))
    label = np.asarray(label)
    assert feature.shape == (N, C), feature.shape
    assert label.shape == (N,), label.shape

    lab16 = label.astype(np.uint16)

    in_maps = []
    for k in range(N_CORES):
        fshard = feature[k * ROWS : (k + 1) * ROWS]
        # all labels, rotated so this core's shard occupies positions 0..2047,
        # then column-major: [p, q] = L[q*P + p]
        rot = np.concatenate([lab16[k * ROWS :], lab16[: k * ROWS]])
        lab_cm = np.ascontiguousarray(rot.reshape(Q, P).T)
        in_maps.append(
            {"feature": np.ascontiguousarray(fshard), "label_cm": lab_cm}
        )

    nc = _get_nc()
    trace = bool(int(os.environ.get("KERNEL_TRACE", "0")))
    res = run_bass_kernel_spmd(
        nc,
        in_maps,
        core_ids=list(range(N_CORES)),
        trace=trace,
    )
    LAST_RESULTS = res

    total = 0.0
    for k in range(N_CORES):
        total += float(res.results[k]["out"].sum())
    return np.float32(-total / N)


# revision 17
# speedup vs baseline: 2.3672x; 1.2465x over previous
"""Cost-sensitive focal NLL loss on 8 Trainium2 NeuronCores.

For feature [N, C] logits and label [N] int:
    log_p = log_softmax(feature, axis=1)
    p = exp(log_p); beta = (1 - p)**2
    counts = bincount(label, C); ni = counts[label]; r = ni / N
    alpha = exp(r - 1) / r
    loss = -mean(alpha * beta[i, label[i]] * log_p[i, label[i]])

Each core streams its 2048-row feature shard once through ScalarE exp
with the fused row-sum accumulator; the label-column values exp(x_i)
are then picked out of the exp outputs with a single GPSIMD
indirect_copy (an SBUF-side per-partition gather - no DMA traffic, so
the feature stream never stalls behind scattered 4-byte reads).

The global class histogram couples all rows, but instead of an
AllReduce (a cross-core barrier costs ~40us of launch skew plus ~20us
collective latency), every core receives ALL 16384 labels (128KB of
uint16, rotated so its own shard's labels are always columns 0..15)
and computes the global histogram redundantly: decompose
c = 32*hi + lo, build bf16 one-hot masks of hi and lo, and accumulate
maskA_g^T @ maskB_g on the PE into a [128,128] PSUM tile whose four
diagonal 32x32 blocks sum to counts[hi, lo]. Zero collectives; each
core's NEFF is fully independent.

A = exp(r-1)/r is computed reciprocal-style (no Ln) so ScalarE keeps
the Exp table until the single ln() at the tail (logp = ln(exp(x)/s)).
Per-row u = beta*logp folds per-class through the same masks; the
final sum_c A_c*U_c uses A pre-replicated onto the diagonal blocks
(A_tiled, zeros elsewhere) so one PSUM*SBUF multiply plus a row
reduction yields a [128,1] partial per core; the host sums 8x128.
"""

import os

import numpy as np

import concourse.bacc as bacc
import concourse.bass as bass
import concourse.mybir as mybir
import concourse.tile as tile
from concourse.bass_utils import run_bass_kernel_spmd

N_CORES = 8
N = 16384
C = 1000
P = 128
ROWS = N // N_CORES          # 2048 rows per core
T = ROWS // P                # 16 row-tiles per core
Q = N // P                   # 128 label chunks of 128 (all cores' labels)
G = Q // 4                   # 32 mask groups of 4 chunks
HL = 32                      # c = 32*hi + lo, hi,lo in [0,32)

FP = mybir.dt.float32
BF = mybir.dt.bfloat16
U16 = mybir.dt.uint16

LAST_RESULTS = None  # BassKernelResults of the most recent run (for profiling)


def build_program(dump_debug: bool = False):
    nc = bacc.Bacc(
        "TRN2",
        target_bir_lowering=False,
        debug=False,
        enable_asserts=False,
        num_devices=N_CORES,
    )

    feature = nc.dram_tensor("feature", [ROWS, C], FP, kind="ExternalInput")
    # all 16384 labels as uint16, column-major [p, q] = L[128q + p]; L is
    # rotated so this core's own shard labels are columns 0..T-1
    label_cm = nc.dram_tensor("label_cm", [P, Q], U16, kind="ExternalInput")
    out = nc.dram_tensor("out", [1, 1], FP, kind="ExternalOutput")
    dbg = {}
    if dump_debug:
        for nm, shape in [
            ("d_cnt", [HL, HL]),
            ("d_xe", [P, T]),
            ("d_s", [P, T]),
            ("d_u", [P, T]),
        ]:
            dbg[nm] = nc.dram_tensor(nm, shape, FP, kind="ExternalOutput")

    with tile.TileContext(nc) as tc:
        with (
            tc.tile_pool(name="const", bufs=1) as const_pool,
            tc.tile_pool(name="feat", bufs=6) as feat_pool,
            tc.tile_pool(name="mask", bufs=1) as mask_pool,
            tc.tile_pool(name="small", bufs=1) as small_pool,
            tc.tile_pool(name="psum", bufs=1, space="PSUM") as psum_pool,
        ):
            # ---- feature stream: issue DMAs first so HBM starts instantly
            fts = []
            for t in range(T):
                ft = feat_pool.tile([P, C], FP, name="ft")
                nc.sync.dma_start(ft[:], feature.ap()[t * P : (t + 1) * P, :])
                fts.append(ft)

            # labels on the gpsimd queue (keeps sync free for the stream)
            lab = small_pool.tile([P, Q], U16)
            nc.gpsimd.dma_start(lab[:], label_cm.ap())

            # ---- constants ----
            iota32_i = const_pool.tile([P, HL], U16)
            nc.gpsimd.iota(iota32_i[:], pattern=[[1, HL]], base=0,
                           channel_multiplier=0)
            iota32_f = const_pool.tile([P, HL], FP)
            nc.vector.tensor_copy(iota32_f[:], iota32_i[:])

            # tbase[p, t] = 1000*t (for gather indices into escall)
            tbase = const_pool.tile([P, T], U16)
            nc.gpsimd.iota(tbase[:], pattern=[[C, T]], base=0,
                           channel_multiplier=0)

            neg1_col = const_pool.tile([P, 1], FP)
            nc.vector.memset(neg1_col[:], -1.0)

            ones_col = const_pool.tile([P, 1], FP)
            nc.vector.memset(ones_col[:], 1.0)

            # sel_k[p, h] = (p == 32k + h): selection matrices that pull the
            # four diagonal 32x32 blocks out of a [128,128] product on the PE
            sels = []
            for k in range(4):
                pmk_i = const_pool.tile([P, 1], mybir.dt.int16, name=f"pmk{k}")
                nc.gpsimd.iota(pmk_i[:], pattern=[[1, 1]], base=-32 * k,
                               channel_multiplier=1)
                pmk_f = const_pool.tile([P, 1], FP, name=f"pmkf{k}")
                nc.vector.tensor_copy(pmk_f[:], pmk_i[:])
                sel = const_pool.tile([P, HL], FP, name=f"sel{k}")
                sels.append(sel)
                nc.vector.tensor_scalar(
                    sel[:], iota32_f[:], pmk_f[:], None,
                    op0=mybir.AluOpType.is_equal,
                )

            # ---- exp + fused row-sum; escall holds all 16 exp tiles ----
            s_col = small_pool.tile([P, T], FP)  # per-row sum(exp(logits))
            escall = small_pool.tile([P, T, C], FP)
            for t in range(T):
                nc.scalar.activation(
                    escall[:, t, :],
                    fts[t][:],
                    mybir.ActivationFunctionType.Exp,
                    accum_out=s_col[:, t : t + 1],
                )

            # gather exp(x_i) = escall[p, t, lab[p,t]] in one SBUF-side op
            gidx = small_pool.tile([P, T], U16)
            nc.vector.tensor_tensor(gidx[:], tbase[:], lab[:, 0:T],
                                    op=mybir.AluOpType.add)
            xe = small_pool.tile([P, T], FP)
            nc.gpsimd.indirect_copy(
                xe[:], escall[:].rearrange("p t c -> p (t c)"), gidx[:],
                i_know_ap_gather_is_preferred=True,
            )

            # ---- one-hot masks (bf16) for the global histogram ----
            hi_f = small_pool.tile([P, Q], FP)
            hi_u = small_pool.tile([P, Q], U16)
            nc.vector.tensor_scalar(
                hi_u[:], lab[:], 5, None,
                op0=mybir.AluOpType.logical_shift_right,
            )
            nc.vector.tensor_copy(hi_f[:], hi_u[:])
            lo_f = small_pool.tile([P, Q], FP)
            lo_u = small_pool.tile([P, Q], U16)
            nc.vector.tensor_scalar(
                lo_u[:], lab[:], 31, None,
                op0=mybir.AluOpType.bitwise_and,
            )
            nc.vector.tensor_copy(lo_f[:], lo_u[:])

            # maskB[p, q, j] = (lo(label[p, q]) == j)
            maskB = mask_pool.tile([P, Q, HL], BF)
            nc.vector.tensor_tensor(
                maskB[:],
                lo_f[:].unsqueeze(2).broadcast_to([P, Q, HL]),
                iota32_f[:].unsqueeze(1).broadcast_to([P, Q, HL]),
                op=mybir.AluOpType.is_equal,
            )
            # maskA_g[p, k, h] = (hi(label[p, 4g+k]) == h), one tile per
            # group of 4 chunks (matmul lhsT must be an unsliced tile)
            maskAs = []
            for g in range(G):
                mA = mask_pool.tile([P, 4, HL], BF, name=f"mA{g}")
                maskAs.append(mA)
                nc.vector.tensor_tensor(
                    mA[:],
                    hi_f[:, 4 * g : 4 * g + 4].unsqueeze(2).broadcast_to(
                        [P, 4, HL]
                    ),
                    iota32_f[:].unsqueeze(1).broadcast_to([P, 4, HL]),
                    op=mybir.AluOpType.is_equal,
                )

            # hist_ps[32k+h, 32k'+j] += sum_p maskA_g[p,k,h]*maskB_g[p,k',j]
            # diagonal blocks k==k' hold per-chunk histograms
            hist_ps = psum_pool.tile([P, P], FP)
            for g in range(G):
                nc.tensor.matmul(
                    hist_ps[:],
                    lhsT=maskAs[g][:],
                    rhs=maskB[:, 4 * g : 4 * g + 4, :],
                    start=(g == 0),
                    stop=(g == G - 1),
                )

            # counts[h, j] = sum_k hist_ps[32k+h, 32k+j]. Vector engines have
            # no cross-lane path; pull the diagonal blocks onto partitions
            # 0..31 with selection matmuls (no DMA -> no slow completion
            # semaphores stalling the e1 slot in ScalarE's in-order queue)
            hist_sb = small_pool.tile([P, P], FP)
            nc.vector.tensor_copy(hist_sb[:], hist_ps[:])
            cnt_ps = psum_pool.tile([HL, HL], FP)
            for k in range(4):
                nc.tensor.matmul(
                    cnt_ps[:],
                    lhsT=sels[k][:],
                    rhs=hist_sb[:, k * HL : (k + 1) * HL],
                    start=(k == 0),
                    stop=(k == 3),
                )

            # A = exp(n/N - 1) * N * (1/n), n clamped at 0.5 so absent
            # classes stay finite (their U is 0)
            cntc = small_pool.tile([HL, HL], FP)
            nc.vector.tensor_scalar_max(cntc[:], cnt_ps[:], 0.5)
            rn = small_pool.tile([HL, HL], FP)
            nc.vector.reciprocal(rn[:], cntc[:])
            e1 = small_pool.tile([HL, HL], FP)
            nc.scalar.activation(
                e1[:], cnt_ps[:], mybir.ActivationFunctionType.Exp,
                bias=neg1_col[0:HL, :], scale=1.0 / N,
            )
            aw = small_pool.tile([HL, HL], FP)
            nc.vector.scalar_tensor_tensor(
                aw[:],
                in0=e1[:],
                scalar=float(N),
                in1=rn[:],
                op0=mybir.AluOpType.mult,
                op1=mybir.AluOpType.mult,
            )
            # A_tiled: A on the four diagonal 32x32 blocks, zero elsewhere
            # (built early, off the critical path, via gpsimd-queue DMAs)
            a_tiled = small_pool.tile([P, P], FP)
            nc.vector.memset(a_tiled[:], 0.0)
            for k in range(4):
                nc.gpsimd.dma_start(
                    a_tiled[k * HL : (k + 1) * HL, k * HL : (k + 1) * HL],
                    aw[:],
                )

            # ---- per-row tail ----
            sinv = small_pool.tile([P, T], FP)
            nc.vector.reciprocal(sinv[:], s_col[:])
            pp = small_pool.tile([P, T], FP)  # p = exp(x)/sumexp
            nc.vector.tensor_tensor(pp[:], xe[:], sinv[:],
                                    op=mybir.AluOpType.mult)
            logp = small_pool.tile([P, T], FP)  # ln(p): the only table switch
            nc.scalar.activation(logp[:], pp[:],
                                 mybir.ActivationFunctionType.Ln)
            pm1 = small_pool.tile([P, T], FP)
            nc.vector.tensor_scalar(pm1[:], pp[:], 1.0, None,
                                    op0=mybir.AluOpType.subtract)
            beta = small_pool.tile([P, T], FP)  # (p-1)^2 == (1-p)^2
            nc.vector.tensor_tensor(beta[:], pm1[:], pm1[:],
                                    op=mybir.AluOpType.mult)
            u = small_pool.tile([P, T], FP)
            nc.vector.tensor_tensor(u[:], beta[:], logp[:],
                                    op=mybir.AluOpType.mult)

            # ---- U via the same masks (own labels = chunks 0..15) ----
            u_ps = psum_pool.tile([P, P], FP)
            for g in range(4):
                uA = small_pool.tile([P, 4, HL], BF, name=f"uA{g}")
                nc.vector.tensor_tensor(
                    uA[:],
                    maskAs[g][:],
                    u[:, 4 * g : 4 * g + 4].unsqueeze(2).broadcast_to(
                        [P, 4, HL]
                    ),
                    op=mybir.AluOpType.mult,
                )
                nc.tensor.matmul(
                    u_ps[:],
                    lhsT=uA[:],
                    rhs=maskB[:, 4 * g : 4 * g + 4, :],
                    start=(g == 0),
                    stop=(g == 3),
                )

            # partial = sum_pj A_tiled[p,j] * u_ps[p,j]; off-diagonal garbage
            # in u_ps is zeroed by A_tiled. Reduce to one scalar on-device:
            # a [128,1]-shaped DMA scatters 128 4B packets over 16 DMA
            # engines whose completion semaphores drip for ~6us after the
            # data lands, so ship a single [1,1] value instead.
            au = small_pool.tile([P, P], FP)
            nc.vector.tensor_tensor(au[:], u_ps[:], a_tiled[:],
                                    op=mybir.AluOpType.mult)
            colsum_ps = psum_pool.tile([1, P], FP)
            nc.tensor.matmul(colsum_ps[:], lhsT=ones_col[:], rhs=au[:],
                             start=True, stop=True)
            fin = small_pool.tile([1, 1], FP)
            nc.vector.tensor_reduce(
                fin[:], colsum_ps[:], axis=mybir.AxisListType.X,
                op=mybir.AluOpType.add,
            )
            nc.sync.dma_start(out.ap(), fin[:])

            if dump_debug:
                nc.sync.dma_start(dbg["d_cnt"].ap(), cntc[:])
                nc.sync.dma_start(dbg["d_xe"].ap(), xe[:])
                nc.sync.dma_start(dbg["d_s"].ap(), s_col[:])
                nc.sync.dma_start(dbg["d_u"].ap(), u[:])

    nc.compile()
    return nc


_NC_CACHE = None


def _get_nc():
    global _NC_CACHE
    if _NC_CACHE is None:
        _NC_CACHE = build_program(
            dump_debug=bool(int(os.environ.get("KERNEL_DEBUG", "0")))
        )
    return _NC_CACHE


def kernel(feature: np.ndarray, label: np.ndarray) -> np.ndarray:
    global LAST_RESULTS
    feature = np.ascontiguousarray(np.asarray(feature, dtype=np.float32))
    label = np.asarray(label)
    assert feature.shape == (N, C), feature.shape
    assert label.shape == (N,), label.shape

    lab16 = label.astype(np.uint16)

    in_maps = []
    for k in range(N_CORES):
        fshard = feature[k * ROWS : (k + 1) * ROWS]
        # all labels, rotated so this core's shard occupies positions 0..2047,
        # then column-major: [p, q] = L[q*P + p]
        rot = np.concatenate([lab16[k * ROWS :], lab16[: k * ROWS]])
        lab_cm = np.ascontiguousarray(rot.reshape(Q, P).T)
        in_maps.append(
            {"feature": np.ascontiguousarray(fshard), "label_cm": lab_cm}
        )

    nc = _get_nc()
    trace = bool(int(os.environ.get("KERNEL_TRACE", "0")))
    res = run_bass_kernel_spmd(
        nc,
        in_maps,
        core_ids=list(range(N_CORES)),
        trace=trace,
    )
    LAST_RESULTS = res

    total = 0.0
    for k in range(N_CORES):
        total += float(res.results[k]["out"][0, 0])
    return np.float32(-total / N)
